# revision 2
# baseline (speedup 1.0000x reference)
"""MDRNN2D (4-direction 2D GRU) Trainium2 Bass kernel — bf16 wavefront.

Sharding: 8 cores = 4 scan directions x 2 batch halves (16 each).
Each core runs a wavefront over the 125 anti-diagonals of its (flipped)
63x63 grid. Hidden state for a diagonal lives in SBUF as (128=hid,
cells*16) bf16, cells ordered by row index i; h_up / h_left of the next
diagonal are 16-column-shifted slices of the previous diagonal's buffer.

Each diagonal is split at a FIXED global cell boundary (cell 32) into
two chunk-streams A/B. A(t+1) depends only on A(t); B(t) needs A(t)'s
last cell + B(t-1) — so with subtile dependency tracking the Tile
scheduler pipelines B(t) against A(t+1), halving the serial chain.

Math per cell (z' = 1-z = sigmoid(-(az+bz))):
  h = z'*n + 0.5*s*(1-z'),  s = h_up + h_left
On-path ops: MM(pr) -> ACT r -> DVE v=r*png -> DVE w=v+pnx -> ACT n
-> DVE m=z'*n -> DVE h=m+t1.  Off-path: ACT z', GPS s, GPS p=z'*s,
GPS d=s-p, DVE t1=0.5*d.  All SBUF tiles bf16 (DVE 2x mode); PSUM fp32.
"""

import os

import numpy as np

B, IN, H_IMG, W_IMG, HID = 32, 64, 64, 64, 128
G = 63            # computed grid is (H-1, W-1)
ND = 2 * G - 1    # number of anti-diagonals
SB = 16           # batch per core
STOT = G * G * SB
TOTAL = STOT
GB = 32           # fixed A/B chunk boundary in global cell index

# (i0, i1, ncells, col_offset) per diagonal; cells of diag t are (i, t-i),
# i in [i0, i1], stored as SB consecutive columns per cell, i ascending.
_DIAG = []
_off = 0
for _t in range(ND):
    _i0, _i1 = max(0, _t - (G - 1)), min(_t, G - 1)
    _n = _i1 - _i0 + 1
    _DIAG.append((_i0, _i1, _n, _off))
    _off += _n * SB
assert _off == STOT

_FLIPS = [(False, False), (True, False), (False, True), (True, True)]

_PROG_CACHE = {}


def _build_program():
    import concourse.mybir as mybir
    import concourse.tile as tile
    from concourse import bacc

    f32 = mybir.dt.float32
    bf16 = mybir.dt.bfloat16
    AF = mybir.ActivationFunctionType
    OP = mybir.AluOpType

    nc = bacc.Bacc()
    xd = nc.declare_dram_parameter("xd", [IN, TOTAL], bf16, isOutput=False)
    wxp = nc.declare_dram_parameter("wx", [IN, 3 * HID], bf16, isOutput=False)
    whp = nc.declare_dram_parameter("wh", [HID, 3 * HID], bf16, isOutput=False)
    wh2p = nc.declare_dram_parameter("wh2", [HID, 3 * HID], bf16, isOutput=False)
    # bias columns: 0 = b_r, 1 = -b_z (for z' = sigmoid(-az - b_z)), 2 = b_n
    bp = nc.declare_dram_parameter("bias", [HID, 3], f32, isOutput=False)
    od = nc.declare_dram_parameter("od", [HID, TOTAL], bf16, isOutput=True)

    with tile.TileContext(nc) as tc:
        with (
            tc.tile_pool(name="const", bufs=1) as cpool,
            tc.tile_pool(name="hbuf", bufs=1) as hpool,
            tc.tile_pool(name="xin", bufs=4) as xpool,
            tc.tile_pool(name="ps", bufs=2, space="PSUM") as ppool,
            tc.tile_pool(name="work", bufs=3) as wpool,
        ):
            wx_t = cpool.tile([IN, 3 * HID], bf16, tag="wx")
            wh_t = cpool.tile([HID, 3 * HID], bf16, tag="wh")
            wh2_t = cpool.tile([HID, 3 * HID], bf16, tag="wh2")
            b_t = cpool.tile([HID, 3], f32, tag="b")
            nc.sync.dma_start(out=wx_t[:], in_=wxp[:])
            nc.sync.dma_start(out=wh_t[:], in_=whp[:])
            nc.sync.dma_start(out=wh2_t[:], in_=wh2p[:])
            nc.sync.dma_start(out=b_t[:], in_=bp[:])

            hbufs = []
            for k in range(2):
                hb = hpool.tile([HID, (G + 2) * SB], bf16, tag=f"h{k}",
                                name=f"hring{k}")
                hbufs.append(hb)
                nc.vector.memset(hb[:], 0.0)

            wxg = [wx_t[:, g * HID:(g + 1) * HID] for g in range(3)]
            whg = [wh_t[:, g * HID:(g + 1) * HID] for g in range(3)]
            wh2g = [wh2_t[:, g * HID:(g + 1) * HID] for g in range(3)]

            for t in range(ND):
                i0, i1, n, off = _DIAG[t]
                cur = hbufs[t % 2]
                prev = hbufs[(t - 1) % 2]
                base = off
                cols = n * SB
                x_t = xpool.tile([IN, cols], bf16, tag="xt")
                nc.sync.dma_start(out=x_t[:], in_=xd[:, base:base + cols])
                # fixed global-cell boundary GB: chunk A = cells < GB,
                # chunk B = cells >= GB (A(t+1) then depends only on A(t))
                if i1 < GB or i0 >= GB:
                    chunks = [(0, n)]
                else:
                    chunks = [(0, GB - i0), (GB - i0, n)]
                for (c0, c1) in chunks:
                    w = (c1 - c0) * SB
                    xs = c0 * SB
                    pr = ppool.tile([HID, w], f32, tag="pr")
                    pz = ppool.tile([HID, w], f32, tag="pz")
                    png = ppool.tile([HID, w], f32, tag="png")
                    pnx = ppool.tile([HID, w], f32, tag="pnx")
                    xin = x_t[:, xs:xs + w]
                    # x-only matmuls first: no dependency on the previous
                    # diagonal, keeps PE busy during the previous step's
                    # elementwise tail
                    nc.tensor.matmul(pnx[:], wxg[2], xin, start=True, stop=True)
                    nc.tensor.matmul(pr[:], wxg[0], xin, start=True, stop=False)
                    nc.tensor.matmul(pz[:], wxg[1], xin, start=True, stop=False)

                    ob = (i0 + 1 + c0) * SB
                    up = (i0 + c0) * SB
                    lf = (i0 + 1 + c0) * SB
                    h_up = prev[:, up:up + w]
                    h_left = prev[:, lf:lf + w]

                    # r-gate h-matmuls first: they gate the critical path
                    nc.tensor.matmul(pr[:], whg[0], h_up, start=False, stop=False)
                    nc.tensor.matmul(pr[:], wh2g[0], h_left, start=False, stop=True)
                    nc.tensor.matmul(png[:], whg[2], h_up, start=True, stop=False)
                    nc.tensor.matmul(png[:], wh2g[2], h_left, start=False, stop=True)
                    nc.tensor.matmul(pz[:], whg[1], h_up, start=False, stop=False)
                    nc.tensor.matmul(pz[:], wh2g[1], h_left, start=False, stop=True)

                    # off-path: s = h_up + h_left, z' = sigmoid(-az - b_z),
                    # p = z'*s, d = s - p, t1 = 0.5*d
                    s_t = wpool.tile([HID, w], bf16, tag="s")
                    nc.gpsimd.tensor_add(s_t[:], h_up, h_left)
                    zp_t = wpool.tile([HID, w], bf16, tag="zp")
                    nc.scalar.activation(zp_t[:], pz[:], AF.Sigmoid,
                                         bias=b_t[:, 1:2], scale=-1.0)
                    p_t = wpool.tile([HID, w], bf16, tag="p")
                    nc.gpsimd.tensor_mul(p_t[:], zp_t[:], s_t[:])
                    d_t = wpool.tile([HID, w], bf16, tag="d")
                    nc.gpsimd.tensor_sub(d_t[:], s_t[:], p_t[:])
                    t1_t = wpool.tile([HID, w], bf16, tag="t1")
                    nc.vector.tensor_scalar_mul(t1_t[:], d_t[:], 0.5)

                    # on-path: r -> v -> w -> n -> m -> h
                    r_t = wpool.tile([HID, w], bf16, tag="r")
                    nc.scalar.activation(r_t[:], pr[:], AF.Sigmoid,
                                         bias=b_t[:, 0:1])
                    v_t = wpool.tile([HID, w], bf16, tag="v")
                    nc.vector.tensor_mul(v_t[:], r_t[:], png[:])
                    w_t = wpool.tile([HID, w], bf16, tag="w")
                    nc.vector.tensor_add(w_t[:], v_t[:], pnx[:])
                    n_t = wpool.tile([HID, w], bf16, tag="n")
                    nc.scalar.activation(n_t[:], w_t[:], AF.Tanh,
                                         bias=b_t[:, 2:3])
                    m_t = wpool.tile([HID, w], bf16, tag="m")
                    nc.vector.tensor_mul(m_t[:], zp_t[:], n_t[:])
                    nc.vector.tensor_add(cur[:, ob:ob + w], m_t[:], t1_t[:])

                    nc.sync.dma_start(
                        out=od[:, base + xs:base + xs + w],
                        in_=cur[:, ob:ob + w])

    nc.finalize()
    return nc


def _host_prep(x, Wx, Wh, Wh2, b):
    """Build per-core input maps (8 cores = 4 dirs x 2 batch halves)."""
    import ml_dtypes
    bf16 = ml_dtypes.bfloat16
    xr = np.ascontiguousarray(np.transpose(x, (2, 3, 0, 1))[:G, :G])  # (G,G,B,IN)
    in_maps = []
    for d, (fy, fx) in enumerate(_FLIPS):
        xg = xr[::-1] if fy else xr
        xg = xg[:, ::-1] if fx else xg
        bd = b[d].reshape(3, HID)
        bias = np.stack([bd[0], -bd[1], bd[2]], axis=1).astype(np.float32)
        wx_b = np.ascontiguousarray(Wx[d]).astype(bf16)
        wh_b = np.ascontiguousarray(Wh[d]).astype(bf16)
        wh2_b = np.ascontiguousarray(Wh2[d]).astype(bf16)
        for half in range(2):
            b0 = half * SB
            xh = xg[:, :, b0:b0 + SB]          # (G,G,SB,IN)
            xdiag = np.empty((IN, TOTAL), bf16)
            for t in range(ND):
                i0, i1, n, off = _DIAG[t]
                ii = np.arange(i0, i1 + 1)
                blk = xh[ii, t - ii]           # (n, SB, IN)
                xdiag[:, off:off + n * SB] = blk.reshape(n * SB, IN).T
            in_maps.append({
                "xd": xdiag,
                "wx": wx_b,
                "wh": wh_b,
                "wh2": wh2_b,
                "bias": np.ascontiguousarray(bias),
            })
    return in_maps


def _host_gather(results):
    out_map = np.ones((4, H_IMG, W_IMG, B, HID), np.float32)
    for d, (fy, fx) in enumerate(_FLIPS):
        o = np.empty((G, G, B, HID), np.float32)
        for half in range(2):
            od = np.asarray(results[d * 2 + half]["od"], np.float32)  # (HID, TOTAL)
            b0 = half * SB
            for t in range(ND):
                i0, i1, n, off = _DIAG[t]
                sl = od[:, off:off + n * SB]
                blk = sl.T.reshape(n, SB, HID)
                ii = np.arange(i0, i1 + 1)
                o[ii, t - ii, b0:b0 + SB] = blk
        o = o[::-1] if fy else o
        o = o[:, ::-1] if fx else o
        oy, ox = (1 if fy else 0), (1 if fx else 0)
        out_map[d, oy:oy + G, ox:ox + G] = o
    return np.ascontiguousarray(np.transpose(out_map, (3, 4, 0, 1, 2)))


def kernel(x, Wx, Wh, Wh2, b):
    from concourse.bass_utils import run_bass_kernel_spmd

    if "prog" not in _PROG_CACHE:
        _PROG_CACHE["prog"] = _build_program()
    nc = _PROG_CACHE["prog"]

    in_maps = _host_prep(
        np.asarray(x, np.float32), np.asarray(Wx, np.float32),
        np.asarray(Wh, np.float32), np.asarray(Wh2, np.float32),
        np.asarray(b, np.float32))
    trace = os.environ.get("MDRNN_TRACE", "0") == "1"
    res = run_bass_kernel_spmd(nc, in_maps, list(range(8)), trace=trace)
    out = _host_gather(res.results)
    if trace:
        kernel.last_exec_time_ns = res.exec_time_ns
        kernel.last_profile = res
    return out


# revision 3
# speedup vs baseline: 1.2504x; 1.2504x over previous
"""MDRNN2D (4-direction 2D GRU) Trainium2 Bass kernel — bf16 wavefront.

Sharding: 8 cores = 4 scan directions x 2 batch halves (16 each).
Each core runs a wavefront over the 125 anti-diagonals of its (flipped)
63x63 grid. Hidden state for a diagonal lives in SBUF as (128=hid,
cells*16) bf16, cells ordered by row index i; h_up / h_left of the next
diagonal are 16-column-shifted slices of the previous diagonal's buffer.

Each diagonal is split at a FIXED global cell boundary (cell 32) into
two chunk-streams A/B. A(t+1) depends only on A(t); B(t) needs A(t)'s
last cell + B(t-1) — so with subtile dependency tracking the Tile
scheduler pipelines B(t) against A(t+1), halving the serial chain.

Math per cell (z' = 1-z = sigmoid(-(az+bz))):
  h = z'*n + 0.5*s*(1-z'),  s = h_up + h_left
On-path ops: MM(pr) -> ACT r -> DVE v=r*png -> DVE w=v+pnx -> ACT n
-> DVE m=z'*n -> DVE h=m+t1.  Off-path: ACT z', GPS s, GPS p=z'*s,
GPS d=s-p, DVE t1=0.5*d.  All SBUF tiles bf16 (DVE 2x mode); PSUM fp32.
"""

import os

import numpy as np

B, IN, H_IMG, W_IMG, HID = 32, 64, 64, 64, 128
G = 63            # computed grid is (H-1, W-1)
ND = 2 * G - 1    # number of anti-diagonals
SB = 16           # batch per core
STOT = G * G * SB
TOTAL = STOT
GB = 32           # fixed A/B chunk boundary in global cell index

# (i0, i1, ncells, col_offset) per diagonal; cells of diag t are (i, t-i),
# i in [i0, i1], stored as SB consecutive columns per cell, i ascending.
_DIAG = []
_off = 0
for _t in range(ND):
    _i0, _i1 = max(0, _t - (G - 1)), min(_t, G - 1)
    _n = _i1 - _i0 + 1
    _DIAG.append((_i0, _i1, _n, _off))
    _off += _n * SB
assert _off == STOT

_FLIPS = [(False, False), (True, False), (False, True), (True, True)]

_PROG_CACHE = {}


def _build_program():
    import concourse.mybir as mybir
    import concourse.tile as tile
    from concourse import bacc

    f32 = mybir.dt.float32
    bf16 = mybir.dt.bfloat16
    AF = mybir.ActivationFunctionType
    OP = mybir.AluOpType

    nc = bacc.Bacc()
    xd = nc.declare_dram_parameter("xd", [IN, TOTAL], bf16, isOutput=False)
    wxp = nc.declare_dram_parameter("wx", [IN, 3 * HID], bf16, isOutput=False)
    whp = nc.declare_dram_parameter("wh", [HID, 3 * HID], bf16, isOutput=False)
    wh2p = nc.declare_dram_parameter("wh2", [HID, 3 * HID], bf16, isOutput=False)
    # bias columns: 0 = b_r, 1 = -b_z (for z' = sigmoid(-az - b_z)), 2 = b_n
    bp = nc.declare_dram_parameter("bias", [HID, 3], f32, isOutput=False)
    od = nc.declare_dram_parameter("od", [HID, TOTAL], bf16, isOutput=True)

    with tile.TileContext(nc) as tc:
        with (
            tc.tile_pool(name="const", bufs=1) as cpool,
            tc.tile_pool(name="hbuf", bufs=1) as hpool,
            tc.tile_pool(name="xin", bufs=4) as xpool,
            tc.tile_pool(name="ps", bufs=2, space="PSUM") as ppool,
            tc.tile_pool(name="work", bufs=3) as wpool,
        ):
            wx_t = cpool.tile([IN, 3 * HID], bf16, tag="wx")
            wh_t = cpool.tile([HID, 3 * HID], bf16, tag="wh")
            wh2_t = cpool.tile([HID, 3 * HID], bf16, tag="wh2")
            b_t = cpool.tile([HID, 3], f32, tag="b")
            nc.sync.dma_start(out=wx_t[:], in_=wxp[:])
            nc.sync.dma_start(out=wh_t[:], in_=whp[:])
            nc.sync.dma_start(out=wh2_t[:], in_=wh2p[:])
            nc.sync.dma_start(out=b_t[:], in_=bp[:])

            hbufs = []
            for k in range(2):
                hb = hpool.tile([HID, (G + 2) * SB], bf16, tag=f"h{k}",
                                name=f"hring{k}")
                hbufs.append(hb)
                nc.vector.memset(hb[:], 0.0)

            wxg = [wx_t[:, g * HID:(g + 1) * HID] for g in range(3)]
            whg = [wh_t[:, g * HID:(g + 1) * HID] for g in range(3)]
            wh2g = [wh2_t[:, g * HID:(g + 1) * HID] for g in range(3)]

            for t in range(ND):
                i0, i1, n, off = _DIAG[t]
                cur = hbufs[t % 2]
                prev = hbufs[(t - 1) % 2]
                base = off
                cols = n * SB
                x_t = xpool.tile([IN, cols], bf16, tag="xt")
                nc.sync.dma_start(out=x_t[:], in_=xd[:, base:base + cols])
                # fixed global-cell boundary GB: chunk A = cells < GB,
                # chunk B = cells >= GB (A(t+1) then depends only on A(t))
                if i1 < GB or i0 >= GB:
                    chunks = [(0, n)]
                else:
                    chunks = [(0, GB - i0), (GB - i0, n)]
                for (c0, c1) in chunks:
                    w = (c1 - c0) * SB
                    xs = c0 * SB
                    pr = ppool.tile([HID, w], f32, tag="pr")
                    pz = ppool.tile([HID, w], f32, tag="pz")
                    png = ppool.tile([HID, w], f32, tag="png")
                    pnx = ppool.tile([HID, w], f32, tag="pnx")
                    xin = x_t[:, xs:xs + w]
                    # x-only matmuls first: no dependency on the previous
                    # diagonal, keeps PE busy during the previous step's
                    # elementwise tail
                    nc.tensor.matmul(pnx[:], wxg[2], xin, start=True, stop=True)
                    nc.tensor.matmul(pr[:], wxg[0], xin, start=True, stop=False)
                    nc.tensor.matmul(pz[:], wxg[1], xin, start=True, stop=False)

                    ob = (i0 + 1 + c0) * SB
                    up = (i0 + c0) * SB
                    lf = (i0 + 1 + c0) * SB
                    h_up = prev[:, up:up + w]
                    h_left = prev[:, lf:lf + w]

                    # r-gate h-matmuls first (they gate the critical path),
                    # then n-gate, z-gate last (z' has slack via ACT order)
                    nc.tensor.matmul(pr[:], whg[0], h_up, start=False, stop=False)
                    nc.tensor.matmul(pr[:], wh2g[0], h_left, start=False, stop=True)
                    nc.tensor.matmul(png[:], whg[2], h_up, start=True, stop=False)
                    nc.tensor.matmul(png[:], wh2g[2], h_left, start=False, stop=True)
                    nc.tensor.matmul(pz[:], whg[1], h_up, start=False, stop=False)
                    nc.tensor.matmul(pz[:], wh2g[1], h_left, start=False, stop=True)

                    # off-path: s = h_up + h_left on gpsimd (ready early)
                    s_t = wpool.tile([HID, w], bf16, tag="s")
                    nc.gpsimd.tensor_add(s_t[:], h_up, h_left)

                    # ACT queue order: r (on-path), z' (feeds tail prep), n
                    r_t = wpool.tile([HID, w], bf16, tag="r")
                    nc.scalar.activation(r_t[:], pr[:], AF.Sigmoid,
                                         bias=b_t[:, 0:1])
                    zp_t = wpool.tile([HID, w], bf16, tag="zp")
                    nc.scalar.activation(zp_t[:], pz[:], AF.Sigmoid,
                                         bias=b_t[:, 1:2], scale=-1.0)

                    # DVE queue: v, w (on-path), p, d, t1 (tail prep), m, h
                    v_t = wpool.tile([HID, w], bf16, tag="v")
                    nc.vector.tensor_mul(v_t[:], r_t[:], png[:])
                    w_t = wpool.tile([HID, w], bf16, tag="w")
                    nc.vector.tensor_add(w_t[:], v_t[:], pnx[:])
                    p_t = wpool.tile([HID, w], bf16, tag="p")
                    nc.vector.tensor_mul(p_t[:], zp_t[:], s_t[:])
                    d_t = wpool.tile([HID, w], bf16, tag="d")
                    nc.vector.tensor_sub(d_t[:], s_t[:], p_t[:])
                    t1_t = wpool.tile([HID, w], bf16, tag="t1")
                    nc.vector.tensor_scalar_mul(t1_t[:], d_t[:], 0.5)

                    n_t = wpool.tile([HID, w], bf16, tag="n")
                    nc.scalar.activation(n_t[:], w_t[:], AF.Tanh,
                                         bias=b_t[:, 2:3])
                    m_t = wpool.tile([HID, w], bf16, tag="m")
                    nc.vector.tensor_mul(m_t[:], zp_t[:], n_t[:])
                    nc.vector.tensor_add(cur[:, ob:ob + w], m_t[:], t1_t[:])

                    nc.sync.dma_start(
                        out=od[:, base + xs:base + xs + w],
                        in_=cur[:, ob:ob + w])

    nc.finalize()
    return nc


def _host_prep(x, Wx, Wh, Wh2, b):
    """Build per-core input maps (8 cores = 4 dirs x 2 batch halves)."""
    import ml_dtypes
    bf16 = ml_dtypes.bfloat16
    xr = np.ascontiguousarray(np.transpose(x, (2, 3, 0, 1))[:G, :G])  # (G,G,B,IN)
    in_maps = []
    for d, (fy, fx) in enumerate(_FLIPS):
        xg = xr[::-1] if fy else xr
        xg = xg[:, ::-1] if fx else xg
        bd = b[d].reshape(3, HID)
        bias = np.stack([bd[0], -bd[1], bd[2]], axis=1).astype(np.float32)
        wx_b = np.ascontiguousarray(Wx[d]).astype(bf16)
        wh_b = np.ascontiguousarray(Wh[d]).astype(bf16)
        wh2_b = np.ascontiguousarray(Wh2[d]).astype(bf16)
        for half in range(2):
            b0 = half * SB
            xh = xg[:, :, b0:b0 + SB]          # (G,G,SB,IN)
            xdiag = np.empty((IN, TOTAL), bf16)
            for t in range(ND):
                i0, i1, n, off = _DIAG[t]
                ii = np.arange(i0, i1 + 1)
                blk = xh[ii, t - ii]           # (n, SB, IN)
                xdiag[:, off:off + n * SB] = blk.reshape(n * SB, IN).T
            in_maps.append({
                "xd": xdiag,
                "wx": wx_b,
                "wh": wh_b,
                "wh2": wh2_b,
                "bias": np.ascontiguousarray(bias),
            })
    return in_maps


def _host_gather(results):
    out_map = np.ones((4, H_IMG, W_IMG, B, HID), np.float32)
    for d, (fy, fx) in enumerate(_FLIPS):
        o = np.empty((G, G, B, HID), np.float32)
        for half in range(2):
            od = np.asarray(results[d * 2 + half]["od"], np.float32)  # (HID, TOTAL)
            b0 = half * SB
            for t in range(ND):
                i0, i1, n, off = _DIAG[t]
                sl = od[:, off:off + n * SB]
                blk = sl.T.reshape(n, SB, HID)
                ii = np.arange(i0, i1 + 1)
                o[ii, t - ii, b0:b0 + SB] = blk
        o = o[::-1] if fy else o
        o = o[:, ::-1] if fx else o
        oy, ox = (1 if fy else 0), (1 if fx else 0)
        out_map[d, oy:oy + G, ox:ox + G] = o
    return np.ascontiguousarray(np.transpose(out_map, (3, 4, 0, 1, 2)))


def kernel(x, Wx, Wh, Wh2, b):
    from concourse.bass_utils import run_bass_kernel_spmd

    if "prog" not in _PROG_CACHE:
        _PROG_CACHE["prog"] = _build_program()
    nc = _PROG_CACHE["prog"]

    in_maps = _host_prep(
        np.asarray(x, np.float32), np.asarray(Wx, np.float32),
        np.asarray(Wh, np.float32), np.asarray(Wh2, np.float32),
        np.asarray(b, np.float32))
    trace = os.environ.get("MDRNN_TRACE", "0") == "1"
    res = run_bass_kernel_spmd(nc, in_maps, list(range(8)), trace=trace)
    out = _host_gather(res.results)
    if trace:
        kernel.last_exec_time_ns = res.exec_time_ns
        kernel.last_profile = res
    return out


# revision 5
# speedup vs baseline: 1.3254x; 1.0600x over previous
"""MDRNN2D (4-direction 2D GRU) Trainium2 Bass kernel — bf16 wavefront.

Sharding: 8 cores = 4 scan directions x 2 batch halves (16 each).
Each core runs a wavefront over the 125 anti-diagonals of its (flipped)
63x63 grid. Hidden state for a diagonal lives in SBUF as (128=hid,
cells*16) bf16, cells ordered by row index i; h_up / h_left of the next
diagonal are 16-column-shifted slices of the previous diagonal's buffer.

Each diagonal is split at a FIXED global cell boundary (cell 32) into
two chunk-streams A/B. A(t+1) depends only on A(t); B(t) needs A(t)'s
last cell + B(t-1) — so with subtile dependency tracking the Tile
scheduler pipelines B(t) against A(t+1), halving the serial chain.

Math per cell (z' = 1-z = sigmoid(-(az+bz))):
  h = z'*n + 0.5*s*(1-z'),  s = h_up + h_left
On-path ops: MM(pr) -> ACT r -> DVE v=r*png -> DVE w=v+pnx -> ACT n
-> DVE m=z'*n -> DVE h=m+t1.  Off-path: ACT z', GPS s, GPS p=z'*s,
GPS d=s-p, DVE t1=0.5*d.  All SBUF tiles bf16 (DVE 2x mode); PSUM fp32.
"""

import os

import numpy as np

B, IN, H_IMG, W_IMG, HID = 32, 64, 64, 64, 128
G = 63            # computed grid is (H-1, W-1)
ND = 2 * G - 1    # number of anti-diagonals
SB = 16           # batch per core
STOT = G * G * SB
TOTAL = STOT
GB = 32           # fixed A/B chunk boundary in global cell index

# (i0, i1, ncells, col_offset) per diagonal; cells of diag t are (i, t-i),
# i in [i0, i1], stored as SB consecutive columns per cell, i ascending.
_DIAG = []
_off = 0
for _t in range(ND):
    _i0, _i1 = max(0, _t - (G - 1)), min(_t, G - 1)
    _n = _i1 - _i0 + 1
    _DIAG.append((_i0, _i1, _n, _off))
    _off += _n * SB
assert _off == STOT

_FLIPS = [(False, False), (True, False), (False, True), (True, True)]

_PROG_CACHE = {}


def _build_program():
    import concourse.mybir as mybir
    import concourse.tile as tile
    from concourse import bacc

    f32 = mybir.dt.float32
    bf16 = mybir.dt.bfloat16
    AF = mybir.ActivationFunctionType
    OP = mybir.AluOpType

    nc = bacc.Bacc()
    xd = nc.declare_dram_parameter("xd", [IN, TOTAL], bf16, isOutput=False)
    wxp = nc.declare_dram_parameter("wx", [IN, 3 * HID], bf16, isOutput=False)
    whp = nc.declare_dram_parameter("wh", [HID, 3 * HID], bf16, isOutput=False)
    wh2p = nc.declare_dram_parameter("wh2", [HID, 3 * HID], bf16, isOutput=False)
    # bias columns: 0 = b_r, 1 = -b_z (for z' = sigmoid(-az - b_z)), 2 = b_n
    bp = nc.declare_dram_parameter("bias", [HID, 3], f32, isOutput=False)
    od = nc.declare_dram_parameter("od", [HID, TOTAL], bf16, isOutput=True)

    with tile.TileContext(nc) as tc:
        with (
            tc.tile_pool(name="const", bufs=1) as cpool,
            tc.tile_pool(name="hbuf", bufs=1) as hpool,
            tc.tile_pool(name="xin", bufs=4) as xpool,
            tc.tile_pool(name="ps", bufs=2, space="PSUM") as ppool,
            tc.tile_pool(name="work", bufs=3) as wpool,
        ):
            wx_t = cpool.tile([IN, 3 * HID], bf16, tag="wx")
            wh_t = cpool.tile([HID, 3 * HID], bf16, tag="wh")
            wh2_t = cpool.tile([HID, 3 * HID], bf16, tag="wh2")
            b_t = cpool.tile([HID, 3], f32, tag="b")
            nc.sync.dma_start(out=wx_t[:], in_=wxp[:])
            nc.sync.dma_start(out=wh_t[:], in_=whp[:])
            nc.sync.dma_start(out=wh2_t[:], in_=wh2p[:])
            nc.sync.dma_start(out=b_t[:], in_=bp[:])

            hbufs = []
            for k in range(2):
                hb = hpool.tile([HID, (G + 2) * SB], bf16, tag=f"h{k}",
                                name=f"hring{k}")
                hbufs.append(hb)
                nc.vector.memset(hb[:], 0.0)

            wxg = [wx_t[:, g * HID:(g + 1) * HID] for g in range(3)]
            whg = [wh_t[:, g * HID:(g + 1) * HID] for g in range(3)]
            wh2g = [wh2_t[:, g * HID:(g + 1) * HID] for g in range(3)]

            for t in range(ND):
                i0, i1, n, off = _DIAG[t]
                cur = hbufs[t % 2]
                prev = hbufs[(t - 1) % 2]
                base = off
                cols = n * SB
                x_t = xpool.tile([IN, cols], bf16, tag="xt")
                nc.sync.dma_start(out=x_t[:], in_=xd[:, base:base + cols])
                # fixed global-cell boundary GB: chunk A = cells < GB,
                # chunk B = cells >= GB (A(t+1) then depends only on A(t))
                if i1 < GB or i0 >= GB:
                    chunks = [(0, n)]
                else:
                    chunks = [(0, GB - i0), (GB - i0, n)]
                for (c0, c1) in chunks:
                    w = (c1 - c0) * SB
                    xs = c0 * SB
                    pr = ppool.tile([HID, w], f32, tag="pr")
                    pz = ppool.tile([HID, w], f32, tag="pz")
                    png = ppool.tile([HID, w], f32, tag="png")
                    pnx = ppool.tile([HID, w], f32, tag="pnx")
                    xin = x_t[:, xs:xs + w]
                    # x-only matmuls first: no dependency on the previous
                    # diagonal, keeps PE busy during the previous step's
                    # elementwise tail
                    nc.tensor.matmul(pnx[:], wxg[2], xin, start=True, stop=True)
                    nc.tensor.matmul(pr[:], wxg[0], xin, start=True, stop=False)
                    nc.tensor.matmul(pz[:], wxg[1], xin, start=True, stop=False)

                    ob = (i0 + 1 + c0) * SB
                    up = (i0 + c0) * SB
                    lf = (i0 + 1 + c0) * SB
                    h_up = prev[:, up:up + w]
                    h_left = prev[:, lf:lf + w]

                    # r-gate h-matmuls first (they gate the critical path),
                    # then n-gate, z-gate last (z' has slack via ACT order)
                    nc.tensor.matmul(pr[:], whg[0], h_up, start=False, stop=False)
                    nc.tensor.matmul(pr[:], wh2g[0], h_left, start=False, stop=True)
                    nc.tensor.matmul(png[:], whg[2], h_up, start=True, stop=False)
                    nc.tensor.matmul(png[:], wh2g[2], h_left, start=False, stop=True)
                    nc.tensor.matmul(pz[:], whg[1], h_up, start=False, stop=False)
                    nc.tensor.matmul(pz[:], wh2g[1], h_left, start=False, stop=True)

                    # off-path: s = h_up + h_left (DVE — gpsimd shares SBUF
                    # ports with DVE and poisons its throughput)
                    s_t = wpool.tile([HID, w], bf16, tag="s")
                    nc.vector.tensor_add(s_t[:], h_up, h_left)

                    # ACT queue order: r (on-path), z' (feeds tail prep), n
                    r_t = wpool.tile([HID, w], bf16, tag="r")
                    nc.scalar.activation(r_t[:], pr[:], AF.Sigmoid,
                                         bias=b_t[:, 0:1])
                    zp_t = wpool.tile([HID, w], bf16, tag="zp")
                    nc.scalar.activation(zp_t[:], pz[:], AF.Sigmoid,
                                         bias=b_t[:, 1:2], scale=-1.0)

                    # DVE queue: v, w (on-path), p, d, t1 (tail prep), m, h
                    v_t = wpool.tile([HID, w], bf16, tag="v")
                    nc.vector.tensor_mul(v_t[:], r_t[:], png[:])
                    w_t = wpool.tile([HID, w], bf16, tag="w")
                    nc.vector.tensor_add(w_t[:], v_t[:], pnx[:])
                    p_t = wpool.tile([HID, w], bf16, tag="p")
                    nc.vector.tensor_mul(p_t[:], zp_t[:], s_t[:])
                    d_t = wpool.tile([HID, w], bf16, tag="d")
                    nc.vector.tensor_sub(d_t[:], s_t[:], p_t[:])
                    t1_t = wpool.tile([HID, w], bf16, tag="t1")
                    nc.vector.tensor_scalar_mul(t1_t[:], d_t[:], 0.5)

                    n_t = wpool.tile([HID, w], bf16, tag="n")
                    nc.scalar.activation(n_t[:], w_t[:], AF.Tanh,
                                         bias=b_t[:, 2:3])
                    m_t = wpool.tile([HID, w], bf16, tag="m")
                    nc.vector.tensor_mul(m_t[:], zp_t[:], n_t[:])
                    nc.vector.tensor_add(cur[:, ob:ob + w], m_t[:], t1_t[:])

                    nc.sync.dma_start(
                        out=od[:, base + xs:base + xs + w],
                        in_=cur[:, ob:ob + w])

                    # PE warmth filler: dummy matmuls into the already-
                    # consumed pnx bank keep HAM from re-throttling to
                    # K=4/8 while the elementwise tail runs (write-after-
                    # read on pnx orders them into the PE-idle window).
                    if (c0, c1) == chunks[-1]:
                        wtot = n * SB
                        ndum = max(0, min(8, int((3000 - 3.75 * wtot) / 213)))
                        dw = min(w, 512)
                        for _ in range(ndum):
                            nc.tensor.matmul(
                                pnx[:, 0:dw], whg[0], prev[:, 0:dw],
                                start=True, stop=True, skip_group_check=True)

    nc.finalize()
    return nc


def _host_prep(x, Wx, Wh, Wh2, b):
    """Build per-core input maps (8 cores = 4 dirs x 2 batch halves)."""
    import ml_dtypes
    bf16 = ml_dtypes.bfloat16
    xr = np.ascontiguousarray(np.transpose(x, (2, 3, 0, 1))[:G, :G])  # (G,G,B,IN)
    in_maps = []
    for d, (fy, fx) in enumerate(_FLIPS):
        xg = xr[::-1] if fy else xr
        xg = xg[:, ::-1] if fx else xg
        bd = b[d].reshape(3, HID)
        bias = np.stack([bd[0], -bd[1], bd[2]], axis=1).astype(np.float32)
        wx_b = np.ascontiguousarray(Wx[d]).astype(bf16)
        wh_b = np.ascontiguousarray(Wh[d]).astype(bf16)
        wh2_b = np.ascontiguousarray(Wh2[d]).astype(bf16)
        for half in range(2):
            b0 = half * SB
            xh = xg[:, :, b0:b0 + SB]          # (G,G,SB,IN)
            xdiag = np.empty((IN, TOTAL), bf16)
            for t in range(ND):
                i0, i1, n, off = _DIAG[t]
                ii = np.arange(i0, i1 + 1)
                blk = xh[ii, t - ii]           # (n, SB, IN)
                xdiag[:, off:off + n * SB] = blk.reshape(n * SB, IN).T
            in_maps.append({
                "xd": xdiag,
                "wx": wx_b,
                "wh": wh_b,
                "wh2": wh2_b,
                "bias": np.ascontiguousarray(bias),
            })
    return in_maps


def _host_gather(results):
    out_map = np.ones((4, H_IMG, W_IMG, B, HID), np.float32)
    for d, (fy, fx) in enumerate(_FLIPS):
        o = np.empty((G, G, B, HID), np.float32)
        for half in range(2):
            od = np.asarray(results[d * 2 + half]["od"], np.float32)  # (HID, TOTAL)
            b0 = half * SB
            for t in range(ND):
                i0, i1, n, off = _DIAG[t]
                sl = od[:, off:off + n * SB]
                blk = sl.T.reshape(n, SB, HID)
                ii = np.arange(i0, i1 + 1)
                o[ii, t - ii, b0:b0 + SB] = blk
        o = o[::-1] if fy else o
        o = o[:, ::-1] if fx else o
        oy, ox = (1 if fy else 0), (1 if fx else 0)
        out_map[d, oy:oy + G, ox:ox + G] = o
    return np.ascontiguousarray(np.transpose(out_map, (3, 4, 0, 1, 2)))


def kernel(x, Wx, Wh, Wh2, b):
    from concourse.bass_utils import run_bass_kernel_spmd

    if "prog" not in _PROG_CACHE:
        _PROG_CACHE["prog"] = _build_program()
    nc = _PROG_CACHE["prog"]

    in_maps = _host_prep(
        np.asarray(x, np.float32), np.asarray(Wx, np.float32),
        np.asarray(Wh, np.float32), np.asarray(Wh2, np.float32),
        np.asarray(b, np.float32))
    trace = os.environ.get("MDRNN_TRACE", "0") == "1"
    res = run_bass_kernel_spmd(nc, in_maps, list(range(8)), trace=trace)
    out = _host_gather(res.results)
    if trace:
        kernel.last_exec_time_ns = res.exec_time_ns
        kernel.last_profile = res
    return out
